# revision 29
# baseline (speedup 1.0000x reference)
"""nn_AttentionAverageStdScalingModule — Trainium2 Bass/Tile kernel.

Contract: kernel(**inputs) takes FULL unsharded inputs and returns the FULL
output (1, 16, 88, 88) f32.  The nseq axis (16) is sharded 2-per-core across
8 NeuronCores; each core runs an identical program on its 2 sequences.

Per sequence s:
  te_n = softmax_temp * test_feat[:,s]/||cols||             (256, 484)
  for each memory m: simT[j,i] = sum_c tr[c,j]*te_n[c,i]    (484j, 484i)
      ez = exp(simT * rsqrt(nsq_j))   <- per-partition scale on ScalarE
      [num;den][i] = [labels_down_m; ones]^T @ ez           (2, 484)
  pmt_down = num/den; pmt = UP @ pmt_down @ UP^T  (bilinear upsample)
  mean/unbiased-std over m, certainty = exp(A/(1+std^2)-A)
  out = certainty*mean + test_scores

Engine split: PE does sim/aggregation/norm-sums/resampling matmuls (bf16),
ScalarE does only Exp (no LUT switches), VectorE does squares/stats and a
bit-trick rsqrt (Newton x2), GpSimd does the fp32->bf16 casts.  Memory-
sums land 4-memories-per-PSUM-bank at 32-aligned partitions so copies and
DRAM relayout bounces are batched.
"""

import numpy as np

NMEM, NSEQ, C, WF, HF = 30, 16, 256, 22, 22
WL, HL = 88, 88
P2 = WF * HF            # 484
NCORES = 8
SEQ_LOC = NSEQ // NCORES  # 2
ALPHA = 20.0
JC = [128, 128, 128, 100]   # j-chunk sizes covering 484
G4 = [list(range(4 * g, min(4 * g + 4, NMEM))) for g in range(8)]
# rsqrt batches (in units of g4 groups): ramp up so exp can start early
BATCHES = [[0], [1, 2], [3, 4], [5, 6, 7]]


def _resize_matrix(n_in: int, n_out: int) -> np.ndarray:
    """Row-stochastic 1-D bilinear resize matrix (half-pixel centers,
    out-of-range taps dropped + renormalized) matching
    jax.image.resize(method='bilinear', antialias=False)."""
    M = np.zeros((n_out, n_in), np.float64)
    scale = n_in / n_out
    for i in range(n_out):
        x = (i + 0.5) * scale - 0.5
        x0 = int(np.floor(x))
        for tap, w in ((x0, 1.0 - (x - x0)), (x0 + 1, x - x0)):
            if 0 <= tap < n_in and w > 0.0:
                M[i, tap] += w
        s = M[i].sum()
        if s > 0:
            M[i] /= s
    return M.astype(np.float32)


_DN = _resize_matrix(WL, WF)   # (22, 88)  downsample
_UP = _resize_matrix(WF, WL)   # (88, 22)  upsample

_CACHE = {}


def _build():
    import concourse.bacc as bacc
    import concourse.mybir as mybir
    from concourse import tile

    f32 = mybir.dt.float32
    bf16 = mybir.dt.bfloat16
    fp8 = mybir.dt.float8e4
    i32 = mybir.dt.int32
    AF = mybir.ActivationFunctionType
    ALU = mybir.AluOpType
    AX = mybir.AxisListType

    nc = bacc.Bacc("TRN2", target_bir_lowering=False, debug=False,
                   num_devices=NCORES)

    t_scores = nc.dram_tensor("t_scores", [SEQ_LOC, WL, HL], f32,
                              kind="ExternalInput")
    t_ldj = nc.dram_tensor("t_ldj", [SEQ_LOC, 128, NMEM, 4, 2], bf16,
                           kind="ExternalInput")
    t_ten = nc.dram_tensor("t_ten", [SEQ_LOC, C, P2], bf16,
                           kind="ExternalInput")
    t_ten8 = nc.dram_tensor("t_ten8", [SEQ_LOC, C, P2], fp8,
                            kind="ExternalInput")
    t_trfeat = nc.dram_tensor("t_trfeat", [NMEM, SEQ_LOC, C, WF, HF], f32,
                              kind="ExternalInput")
    t_temp = nc.dram_tensor("t_temp", [1], f32, kind="ExternalInput")
    t_upt32 = nc.dram_tensor("t_upt32", [WF, WL], f32, kind="ExternalInput")
    t_upt16 = nc.dram_tensor("t_upt16", [WF, WL], bf16, kind="ExternalInput")
    t_ident = nc.dram_tensor("t_ident", [WL, WL], bf16, kind="ExternalInput")
    t_ident32 = nc.dram_tensor("t_ident32", [WL, WL], f32,
                               kind="ExternalInput")
    t_out = nc.dram_tensor("t_out", [SEQ_LOC, WL, HL], f32,
                           kind="ExternalOutput")

    with tile.TileContext(nc) as tc:
        with (
            tc.tile_pool(name="const", bufs=1) as cpool,
            tc.tile_pool(name="seq", bufs=1) as spool,
            tc.tile_pool(name="big", bufs=3) as bpool,
            tc.tile_pool(name="ez", bufs=4) as ezpool,
            tc.tile_pool(name="sm", bufs=2) as smpool,
            tc.tile_pool(name="dram", bufs=1, space="DRAM") as dpool,
            tc.tile_pool(name="psA", bufs=3, space="PSUM") as psA,
            tc.tile_pool(name="psBC", bufs=3, space="PSUM") as psBC,
            tc.tile_pool(name="psD", bufs=1, space="PSUM") as psD,
        ):
            # ---- constants ----
            ones_col32 = cpool.tile([128, 1], f32)
            nc.vector.memset(ones_col32[:], 1.0)
            ones_col16 = cpool.tile([128, 1], bf16)
            nc.vector.memset(ones_col16[:], 1.0)
            ones_row32 = cpool.tile([1, 128], f32)
            nc.vector.memset(ones_row32[:], 1.0)
            upt32 = cpool.tile([WF, WL], f32)      # UP^T (22, 88)
            nc.sync.dma_start(upt32[:], t_upt32[:])
            upt16 = cpool.tile([WF, WL], bf16)
            nc.sync.dma_start(upt16[:], t_upt16[:])
            ident = cpool.tile([WL, WL], bf16)
            nc.sync.dma_start(ident[:], t_ident[:])
            ident32 = cpool.tile([WL, WL], f32)
            nc.sync.dma_start(ident32[:], t_ident32[:])
            temp_t = cpool.tile([1, 1], f32)
            nc.sync.dma_start(temp_t[:], t_temp[:])

            te_n_s, ldj_s, invj_s, labs_s = {}, {}, {}, {}
            nrm_stage_s, nd_stage_s, pd_stage_s = {}, {}, {}
            mean_s, rv_s, ts_s, pdx_s = {}, {}, {}, {}

            # =========== prep for BOTH sequences up front ===========
            for s in range(SEQ_LOC):
                te_n = spool.tile([128, 2, 512], fp8, tag=f"te_n{s}",
                                  name=f"te_n{s}")
                nc.sync.dma_start(
                    te_n[:, :, 0:P2],
                    t_ten8[s].rearrange("(h p) j -> p h j", p=128))
                te_n_s[s] = te_n
                ldj = spool.tile([128, NMEM, 4, 2], bf16,
                                 tag=f"ldj{s}", name=f"ldj{s}")
                nc.sync.dma_start(ldj[:], t_ldj[s])
                ldj_s[s] = ldj
                nrm_stage_s[s] = dpool.tile([NMEM, 512], f32,
                                            tag=f"nrm_stage{s}",
                                            name=f"nrm_stage{s}")
                nd_stage_s[s] = dpool.tile([2, NMEM, P2], f32,
                                           tag=f"nd_stage{s}",
                                           name=f"nd_stage{s}")
                pd_stage_s[s] = dpool.tile([NMEM, P2], f32,
                                           tag=f"pd_stage{s}",
                                           name=f"pd_stage{s}")
                invj_s[s] = spool.tile([128, NMEM, 4], f32, tag=f"invj{s}",
                                       name=f"invj{s}")
                ts = spool.tile([WL, HL], f32, tag=f"ts{s}", name=f"ts{s}")
                nc.sync.dma_start(ts[:], t_scores[s])
                ts_s[s] = ts

            # =========== flat 16-group pipeline across both seqs ======
            trbf_t, sq_t, tr8_t = {}, {}, {}
            agp_t = {}
            pending = [None]
            LA = 2
            STEPS = [(s, g) for g in range(len(G4)) for s in range(SEQ_LOC)]

            def phase1(s, g):
                nrm_stage, invj = nrm_stage_s[s], invj_s[s]
                nsqp = psBC.tile([128, P2], f32, tag="psbc",
                                 name=f"nsqp_{s}_{g}")
                gm = G4[g]
                pairs = [gm[i:i + 2] for i in range(0, len(gm), 2)]
                for pr in pairs:
                    trp2 = bpool.tile([128, 2, 2, P2], bf16,
                                      tag="trbf", bufs=10,
                                      name=f"trp2_{s}_{pr[0]}")
                    for k, m in enumerate(pr):
                        nc.gpsimd.dma_start(
                            trp2[:, k, :, :],
                            t_trfeat[m, s]
                            .rearrange("(h p) w hh -> p h (w hh)", p=128))
                        trbf_t[(s, m)] = trp2[:, k, :, :]
                    tr8 = bpool.tile([128, 2, 2, 512], fp8, tag="tr8",
                                     bufs=10, name=f"tr8_{s}_{pr[0]}")
                    nc.gpsimd.dma_start(tr8[:, :, :, 0:P2], trp2[:])
                    for k, m in enumerate(pr):
                        tr8_t[(s, m)] = tr8[:, k, :, :]
                    sq2 = bpool.tile([128, 2, 2, P2], bf16, tag="sqbf",
                                     name=f"sq2_{s}_{pr[0]}")
                    nc.vector.tensor_tensor(
                        out=sq2[:], in0=trp2[:], in1=trp2[:], op=ALU.mult)
                    for k, m in enumerate(pr):
                        sq_t[(s, m)] = sq2[:, k, :, :]
                for h in range(2):
                    for m in gm:
                        r = 32 * (m % 4)
                        nc.tensor.matmul(
                            nsqp[r:r + 1, :], ones_col16[:],
                            sq_t[(s, m)][:, h, :],
                            start=(h == 0), stop=(h == 1),
                            tile_position=(0, r))
                for m in gm:
                    sq_t.pop((s, m), None)
                nsqsb = smpool.tile([128, P2], f32, tag="nsqsb")
                nc.vector.tensor_copy(nsqsb[:], nsqp[:])
                m0, nmg = gm[0], len(gm)
                nc.sync.dma_start(
                    nrm_stage[m0:m0 + nmg, 0:P2],
                    nsqsb[0:(nmg - 1) * 32 + 1:32, :])
                nc.sync.dma_start(
                    invj[:, m0:m0 + nmg, :],
                    nrm_stage[m0:m0 + nmg].rearrange("m (q p) -> p m q",
                                                     p=128))
                xv = invj[:, m0:m0 + nmg, :]
                nw = smpool.tile([128, 3, 4, 4], f32, tag="nw",
                                 name=f"nw_{s}_{g}")
                xh = nw[:, 0, 0:nmg, :]
                yv = nw[:, 1, 0:nmg, :]
                tv = nw[:, 2, 0:nmg, :]
                nc.vector.tensor_scalar_mul(xh, xv, 0.5)
                nc.vector.tensor_scalar(
                    out=yv.bitcast(i32), in0=xv.bitcast(i32),
                    scalar1=1, scalar2=None, op0=ALU.logical_shift_right)
                nc.vector.tensor_scalar(
                    out=yv.bitcast(i32), in0=yv.bitcast(i32),
                    scalar1=-1, scalar2=0x5F3759DF,
                    op0=ALU.mult, op1=ALU.add)
                for _ in range(2):
                    nc.vector.tensor_tensor(out=tv, in0=yv, in1=yv,
                                            op=ALU.mult)
                    nc.vector.tensor_tensor(out=tv, in0=tv, in1=xh,
                                            op=ALU.mult)
                    nc.vector.tensor_scalar(
                        out=tv, in0=tv, scalar1=-1.0, scalar2=1.5,
                        op0=ALU.mult, op1=ALU.add)
                    nc.vector.tensor_tensor(out=yv, in0=yv, in1=tv,
                                            op=ALU.mult)
                nc.vector.tensor_copy(xv, yv)

            def emit_aggs(s, g, ezs):
                ldj, nd_stage = ldj_s[s], nd_stage_s[s]
                agp = psBC.tile([128, P2], f32, tag="psbc",
                                name=f"agp_{s}_{g}")
                for q in range(4):
                    pq = JC[q]
                    for m in G4[g]:
                        r = 32 * (m % 4)
                        nc.tensor.matmul(
                            agp[r:r + 2, :], ldj[0:pq, m, q, :],
                            ezs[q][m][0:pq, :],
                            start=(q == 0), stop=(q == 3),
                            tile_position=(0, r))
                ndsb = smpool.tile([128, P2], f32, tag="ndsb")
                nc.vector.tensor_copy(ndsb[:], agp[:])
                m0, nmg = G4[g][0], len(G4[g])
                nc.sync.dma_start(
                    nd_stage[0, m0:m0 + nmg, :],
                    ndsb[0:(nmg - 1) * 32 + 1:32, :])
                nc.sync.dma_start(
                    nd_stage[1, m0:m0 + nmg, :],
                    ndsb[1:(nmg - 1) * 32 + 2:32, :])

            def phase2(s, g):
                te_n, invj = te_n_s[s], invj_s[s]
                ezs = {}
                for q in range(4):
                    pq = JC[q]
                    j0 = 128 * q
                    ez_t = {}
                    for m in G4[g]:
                        tr8 = tr8_t[(s, m)]
                        st = psA.tile([128, P2], f32, tag="psa",
                                      name=f"st_{s}_{g}_{q}_{m}")
                        nc.tensor.matmul(
                            st[0:pq, :], tr8[:, :, j0:j0 + pq],
                            te_n[:, :, 0:P2],
                            perf_mode=mybir.MatmulPerfMode.DoubleRow)
                        ez = ezpool.tile([128, P2], bf16, tag="ez",
                                         bufs=24, name=f"ez_{s}_{g}_{q}_{m}")
                        nc.scalar.activation(
                            ez[0:pq, :], st[0:pq, :], AF.Exp,
                            scale=invj[0:pq, m, q:q + 1])
                        ez_t[m] = ez
                    ezs[q] = ez_t
                if pending[0] is not None:
                    emit_aggs(*pending[0])
                pending[0] = (s, g, ezs)
                for m in G4[g]:
                    trbf_t.pop((s, m), None)
                    tr8_t.pop((s, m), None)

            def division(s):
                nd_stage = nd_stage_s[s]
                numt = spool.tile([121, 120], f32, tag=f"numt{s}",
                                  name=f"numt{s}")
                nc.sync.dma_start(
                    numt[:], nd_stage[0].rearrange("m j -> (m j)")
                    .rearrange("(p x) -> p x", p=121))
                dent = spool.tile([121, 120], f32, tag=f"dent{s}",
                                  name=f"dent{s}")
                nc.sync.dma_start(
                    dent[:], nd_stage[1].rearrange("m j -> (m j)")
                    .rearrange("(p x) -> p x", p=121))
                rden = spool.tile([121, 120], f32, tag=f"rden{s}",
                                  name=f"rden{s}")
                nc.vector.reciprocal(rden[:], dent[:])
                pdq = spool.tile([121, 120], f32, tag=f"pdq{s}",
                                 name=f"pdq{s}")
                nc.vector.tensor_tensor(out=pdq[:], in0=numt[:],
                                        in1=rden[:], op=ALU.mult)
                nc.sync.dma_start(
                    pd_stage_s[s][:].rearrange("m j -> (m j)")
                    .rearrange("(p x) -> p x", p=121), pdq[:])

            def tail(s):
                # read pmt_down as (j_row, m, k_col): 88-byte runs
                pdx = spool.tile([WF, NMEM * WF], f32, tag=f"pdx{s}",
                                 name=f"pdx{s}")
                nc.sync.dma_start(
                    pdx[:],
                    pd_stage_s[s][:].rearrange("m (j k) -> j m k", k=WF))
                d1a = psD.tile([WL, 512], f32, tag="d1a",
                               name=f"d1a_{s}")
                nc.tensor.matmul(d1a[:], upt32[:], pdx[:, 0:512])
                d1b = psD.tile([WL, NMEM * WF - 512], f32, tag="d1b",
                               name=f"d1b_{s}")
                nc.tensor.matmul(d1b[:], upt32[:], pdx[:, 512:])
                d1s = spool.tile([WL, NMEM, WF], bf16, tag="d1s")
                d1f = d1s[:].rearrange("l m j -> l (m j)")
                nc.vector.tensor_copy(d1f[:, 0:512], d1a[:])
                nc.vector.tensor_copy(d1f[:, 512:], d1b[:])
                d1t = spool.tile([WF, NMEM, WL], bf16, tag="d1t")
                s1 = spool.tile([WL, HL], f32, tag="s1", name=f"s1_{s}")
                s2 = spool.tile([WL, HL], f32, tag="s2", name=f"s2_{s}")
                for gg in range(6):
                    m0 = 5 * gg
                    for m in range(m0, m0 + 5):
                        trp = psD.tile([WF, WL], bf16,
                                       tag=("d1a" if m % 2 else "d1b"),
                                       name=f"trp_{s}_{m}")
                        nc.tensor.transpose(trp[:], d1s[:, m, :], ident[:])
                        nc.vector.tensor_copy(d1t[:, m, :], trp[:])
                    d2 = psA.tile([WL, 440], f32, tag="psa",
                                  name=f"d2_{s}_{gg}")
                    nc.tensor.matmul(
                        d2[:], upt16[:],
                        d1t[:, m0:m0 + 5, :].rearrange("k m a -> k (m a)"))
                    d2c = smpool.tile([WL, 440], f32, tag="d2c")
                    nc.vector.tensor_copy(d2c[:], d2[:])
                    d2v = d2c[:].rearrange("b (m a) -> b a m", m=5)
                    sqg = smpool.tile([WL, 440], f32, tag="sqg")
                    nc.vector.tensor_tensor(out=sqg[:], in0=d2c[:],
                                            in1=d2c[:], op=ALU.mult)
                    if gg == 0:
                        nc.vector.tensor_reduce(
                            s1[:], d2v, axis=AX.X, op=ALU.add)
                        nc.vector.tensor_reduce(
                            s2[:], sqg[:].rearrange("b (m a) -> b a m", m=5),
                            axis=AX.X, op=ALU.add)
                    else:
                        p1 = smpool.tile([WL, HL], f32, tag="p1")
                        nc.vector.tensor_reduce(
                            p1[:], d2v, axis=AX.X, op=ALU.add)
                        nc.vector.tensor_tensor(out=s1[:], in0=s1[:],
                                                in1=p1[:], op=ALU.add)
                        p2 = smpool.tile([WL, HL], f32, tag="p2")
                        nc.vector.tensor_reduce(
                            p2[:], sqg[:].rearrange("b (m a) -> b a m", m=5),
                            axis=AX.X, op=ALU.add)
                        nc.vector.tensor_tensor(out=s2[:], in0=s2[:],
                                                in1=p2[:], op=ALU.add)

                mean = spool.tile([WL, HL], f32, tag=f"mean{s}",
                                  name=f"mean{s}")
                nc.vector.tensor_scalar_mul(mean[:], s1[:], 1.0 / NMEM)
                ms = spool.tile([WL, HL], f32, tag="ms")
                nc.vector.tensor_tensor(out=ms[:], in0=mean[:], in1=mean[:],
                                        op=ALU.mult)
                v1 = spool.tile([WL, HL], f32, tag="v1")
                nc.vector.tensor_scalar_mul(v1[:], s2[:], 1.0 / (NMEM - 1))
                v2 = spool.tile([WL, HL], f32, tag="v2")
                nc.vector.tensor_scalar_mul(v2[:], ms[:], NMEM / (NMEM - 1.0))
                var = spool.tile([WL, HL], f32, tag="var")
                nc.vector.tensor_tensor(out=var[:], in0=v1[:], in1=v2[:],
                                        op=ALU.subtract)
                vp1 = spool.tile([WL, HL], f32, tag="vp1")
                nc.vector.tensor_scalar_add(vp1[:], var[:], 1.0)
                rv = spool.tile([WL, HL], f32, tag=f"rv{s}", name=f"rv{s}")
                nc.vector.reciprocal(rv[:], vp1[:])
                mean_s[s], rv_s[s] = mean, rv

            warm = spool.tile([1, 1], f32, tag="warm")
            nc.scalar.activation(warm[:], temp_t[:], AF.Exp, scale=0.001)

            NSTEP = len(STEPS)
            for i in range(NSTEP + LA):
                if i < NSTEP:
                    phase1(*STEPS[i])
                if i >= LA:
                    phase2(*STEPS[i - LA])
                if i - LA == 2 * len(G4) - 2:
                    division(0)
            if pending[0] is not None:
                emit_aggs(*pending[0])
                pending[0] = None
            division(1)
            tail(0)
            tail(1)

            # =========== deferred certainty + output ===========
            nalpha = cpool.tile([WL, 1], f32)
            nc.vector.memset(nalpha[:], -ALPHA)
            for s in range(SEQ_LOC):
                cert = spool.tile([WL, HL], f32, tag=f"cert{s}",
                                  name=f"cert{s}")
                nc.scalar.activation(cert[:], rv_s[s][:], AF.Exp,
                                     scale=ALPHA, bias=nalpha[:])
                o1 = spool.tile([WL, HL], f32, tag=f"o1{s}", name=f"o1{s}")
                nc.vector.tensor_tensor(out=o1[:], in0=cert[:],
                                        in1=mean_s[s][:], op=ALU.mult)
                o1p = psD.tile([WL, HL], f32, tag="d1a", name=f"o1p{s}")
                nc.tensor.transpose(o1p[:], o1[:], ident32[:])
                o1t = spool.tile([WL, HL], f32, tag=f"o1t{s}",
                                 name=f"o1t{s}")
                nc.vector.tensor_copy(o1t[:], o1p[:])
                o2 = spool.tile([WL, HL], f32, tag=f"o2{s}", name=f"o2{s}")
                nc.vector.tensor_tensor(out=o2[:], in0=o1t[:],
                                        in1=ts_s[s][:], op=ALU.add)
                nc.sync.dma_start(t_out[s], o2[:])

    nc.compile()
    return nc


def _get_nc():
    if "nc" not in _CACHE:
        _CACHE["nc"] = _build()
    return _CACHE["nc"]


def _bf16(a):
    import ml_dtypes
    return np.ascontiguousarray(a).astype(ml_dtypes.bfloat16)


def _run(test_scores, train_labels, test_feat, train_feats, softmax_temp,
         trace=False):
    from concourse.bass_utils import run_bass_kernel_spmd

    test_scores = np.ascontiguousarray(test_scores, np.float32)
    train_labels = np.ascontiguousarray(train_labels, np.float32)
    test_feat = np.ascontiguousarray(test_feat, np.float32)
    train_feats = np.ascontiguousarray(train_feats, np.float32)
    temp = np.ascontiguousarray(softmax_temp, np.float32).reshape(1)

    te = test_feat[0].reshape(NSEQ, C, P2)
    nrm = np.sqrt((te * te).sum(axis=1, keepdims=True))
    ten = _bf16(temp[0] * te / nrm)
    import concourse.mybir as mybir
    ten8 = (temp[0] * te / nrm).astype(mybir.dt.np(mybir.dt.float8e4))

    lab = train_labels.reshape(NMEM * NSEQ, WL, HL)
    ld = (_DN @ lab @ _DN.T).reshape(NMEM, NSEQ, P2)
    ldj = np.zeros((NSEQ, 128, NMEM, 4, 2), np.float32)
    for q in range(4):
        pq = JC[q]
        ldj[:, 0:pq, :, q, 0] = ld[:, :, 128 * q:128 * q + pq].transpose(
            1, 2, 0)
        ldj[:, 0:pq, :, q, 1] = 1.0
    ldj = _bf16(ldj)

    in_maps = []
    for c in range(NCORES):
        sl = slice(SEQ_LOC * c, SEQ_LOC * (c + 1))
        in_maps.append({
            "t_scores": test_scores[0, sl],
            "t_ldj": ldj[sl],
            "t_ten": ten[sl],
            "t_ten8": ten8[sl],
            "t_trfeat": np.ascontiguousarray(train_feats[:, sl]),
            "t_temp": temp,
            "t_upt32": np.ascontiguousarray(_UP.T),
            "t_upt16": _bf16(_UP.T),
            "t_ident": _bf16(np.eye(WL, dtype=np.float32)),
            "t_ident32": np.eye(WL, dtype=np.float32),
        })
    nc = _get_nc()
    res = run_bass_kernel_spmd(nc, in_maps, list(range(NCORES)), trace=trace)
    out = np.empty((1, NSEQ, WL, HL), np.float32)
    for c in range(NCORES):
        out[0, SEQ_LOC * c:SEQ_LOC * (c + 1)] = res.results[c]["t_out"]
    return out, res


def kernel(test_scores, train_labels, test_feat, train_feats, softmax_temp):
    out, _ = _run(test_scores, train_labels, test_feat, train_feats,
                  softmax_temp, trace=False)
    return out


# revision 30
# speedup vs baseline: 1.0671x; 1.0671x over previous
"""nn_AttentionAverageStdScalingModule — Trainium2 Bass/Tile kernel.

Contract: kernel(**inputs) takes FULL unsharded inputs and returns the FULL
output (1, 16, 88, 88) f32.  The nseq axis (16) is sharded 2-per-core across
8 NeuronCores; each core runs an identical program on its 2 sequences.

Per sequence s:
  te_n = softmax_temp * test_feat[:,s]/||cols||             (256, 484)
  for each memory m: simT[j,i] = sum_c tr[c,j]*te_n[c,i]    (484j, 484i)
      ez = exp(simT * rsqrt(nsq_j))   <- per-partition scale on ScalarE
      [num;den][i] = [labels_down_m; ones]^T @ ez           (2, 484)
  pmt_down = num/den; pmt = UP @ pmt_down @ UP^T  (bilinear upsample)
  mean/unbiased-std over m, certainty = exp(A/(1+std^2)-A)
  out = certainty*mean + test_scores

Engine split: PE does sim/aggregation/norm-sums/resampling matmuls (bf16),
ScalarE does only Exp (no LUT switches), VectorE does squares/stats and a
bit-trick rsqrt (Newton x2), GpSimd does the fp32->bf16 casts.  Memory-
sums land 4-memories-per-PSUM-bank at 32-aligned partitions so copies and
DRAM relayout bounces are batched.
"""

import numpy as np

NMEM, NSEQ, C, WF, HF = 30, 16, 256, 22, 22
WL, HL = 88, 88
P2 = WF * HF            # 484
NCORES = 8
SEQ_LOC = NSEQ // NCORES  # 2
ALPHA = 20.0
JC = [128, 128, 128, 100]   # j-chunk sizes covering 484
G4 = [list(range(4 * g, min(4 * g + 4, NMEM))) for g in range(8)]
# rsqrt batches (in units of g4 groups): ramp up so exp can start early
BATCHES = [[0], [1, 2], [3, 4], [5, 6, 7]]


def _resize_matrix(n_in: int, n_out: int) -> np.ndarray:
    """Row-stochastic 1-D bilinear resize matrix (half-pixel centers,
    out-of-range taps dropped + renormalized) matching
    jax.image.resize(method='bilinear', antialias=False)."""
    M = np.zeros((n_out, n_in), np.float64)
    scale = n_in / n_out
    for i in range(n_out):
        x = (i + 0.5) * scale - 0.5
        x0 = int(np.floor(x))
        for tap, w in ((x0, 1.0 - (x - x0)), (x0 + 1, x - x0)):
            if 0 <= tap < n_in and w > 0.0:
                M[i, tap] += w
        s = M[i].sum()
        if s > 0:
            M[i] /= s
    return M.astype(np.float32)


_DN = _resize_matrix(WL, WF)   # (22, 88)  downsample
_UP = _resize_matrix(WF, WL)   # (88, 22)  upsample

_CACHE = {}


def _build():
    import concourse.bacc as bacc
    import concourse.mybir as mybir
    from concourse import tile

    f32 = mybir.dt.float32
    bf16 = mybir.dt.bfloat16
    fp8 = mybir.dt.float8e4
    i32 = mybir.dt.int32
    AF = mybir.ActivationFunctionType
    ALU = mybir.AluOpType
    AX = mybir.AxisListType

    nc = bacc.Bacc("TRN2", target_bir_lowering=False, debug=False,
                   num_devices=NCORES)

    t_scores = nc.dram_tensor("t_scores", [SEQ_LOC, WL, HL], f32,
                              kind="ExternalInput")
    t_ldj = nc.dram_tensor("t_ldj", [SEQ_LOC, 128, NMEM, 4, 2], bf16,
                           kind="ExternalInput")
    t_ten = nc.dram_tensor("t_ten", [SEQ_LOC, C, P2], bf16,
                           kind="ExternalInput")
    t_ten8 = nc.dram_tensor("t_ten8", [SEQ_LOC, C, P2], fp8,
                            kind="ExternalInput")
    t_trfeat = nc.dram_tensor("t_trfeat", [NMEM, SEQ_LOC, C, WF, HF], f32,
                              kind="ExternalInput")
    t_temp = nc.dram_tensor("t_temp", [1], f32, kind="ExternalInput")
    t_upt32 = nc.dram_tensor("t_upt32", [WF, WL], f32, kind="ExternalInput")
    t_upt16 = nc.dram_tensor("t_upt16", [WF, WL], bf16, kind="ExternalInput")
    t_ident = nc.dram_tensor("t_ident", [WL, WL], bf16, kind="ExternalInput")
    t_ident32 = nc.dram_tensor("t_ident32", [WL, WL], f32,
                               kind="ExternalInput")
    t_out = nc.dram_tensor("t_out", [SEQ_LOC, WL, HL], f32,
                           kind="ExternalOutput")

    with tile.TileContext(nc) as tc:
        with (
            tc.tile_pool(name="const", bufs=1) as cpool,
            tc.tile_pool(name="seq", bufs=1) as spool,
            tc.tile_pool(name="big", bufs=3) as bpool,
            tc.tile_pool(name="ez", bufs=4) as ezpool,
            tc.tile_pool(name="sm", bufs=2) as smpool,
            tc.tile_pool(name="dram", bufs=1, space="DRAM") as dpool,
            tc.tile_pool(name="psA", bufs=4, space="PSUM") as psA,
            tc.tile_pool(name="psBC", bufs=2, space="PSUM") as psBC,
            tc.tile_pool(name="psD", bufs=1, space="PSUM") as psD,
        ):
            # ---- constants ----
            ones_col32 = cpool.tile([128, 1], f32)
            nc.vector.memset(ones_col32[:], 1.0)
            ones_col16 = cpool.tile([128, 1], bf16)
            nc.vector.memset(ones_col16[:], 1.0)
            ones_row32 = cpool.tile([1, 128], f32)
            nc.vector.memset(ones_row32[:], 1.0)
            upt32 = cpool.tile([WF, WL], f32)      # UP^T (22, 88)
            nc.sync.dma_start(upt32[:], t_upt32[:])
            upt16 = cpool.tile([WF, WL], bf16)
            nc.sync.dma_start(upt16[:], t_upt16[:])
            ident = cpool.tile([WL, WL], bf16)
            nc.sync.dma_start(ident[:], t_ident[:])
            ident32 = cpool.tile([WL, WL], f32)
            nc.sync.dma_start(ident32[:], t_ident32[:])
            temp_t = cpool.tile([1, 1], f32)
            nc.sync.dma_start(temp_t[:], t_temp[:])

            te_n_s, ldj_s, invj_s, labs_s = {}, {}, {}, {}
            nrm_stage_s, nd_stage_s, pd_stage_s = {}, {}, {}
            mean_s, rv_s, ts_s, pdx_s = {}, {}, {}, {}

            # =========== prep for BOTH sequences up front ===========
            for s in range(SEQ_LOC):
                te_n = spool.tile([128, 2, P2], bf16, tag=f"te_n{s}",
                                  name=f"te_n{s}")
                nc.sync.dma_start(
                    te_n[:],
                    t_ten[s].rearrange("(h p) j -> p h j", p=128))
                te_n_s[s] = te_n
                ldj = spool.tile([128, NMEM, 4, 2], bf16,
                                 tag=f"ldj{s}", name=f"ldj{s}")
                nc.sync.dma_start(ldj[:], t_ldj[s])
                ldj_s[s] = ldj
                nrm_stage_s[s] = dpool.tile([NMEM, 512], f32,
                                            tag=f"nrm_stage{s}",
                                            name=f"nrm_stage{s}")
                nd_stage_s[s] = dpool.tile([2, NMEM, P2], f32,
                                           tag=f"nd_stage{s}",
                                           name=f"nd_stage{s}")
                pd_stage_s[s] = dpool.tile([NMEM, P2], f32,
                                           tag=f"pd_stage{s}",
                                           name=f"pd_stage{s}")
                invj_s[s] = spool.tile([128, NMEM, 4], f32, tag=f"invj{s}",
                                       name=f"invj{s}")
                ts = spool.tile([WL, HL], f32, tag=f"ts{s}", name=f"ts{s}")
                nc.sync.dma_start(ts[:], t_scores[s])
                ts_s[s] = ts

            # =========== flat 16-group pipeline across both seqs ======
            trbf_t, sq_t, tr8_t = {}, {}, {}
            agp_t = {}
            pending = [None]
            LA = 2
            STEPS = [(s, g) for g in range(len(G4)) for s in range(SEQ_LOC)]

            def phase1(s, g):
                nrm_stage, invj = nrm_stage_s[s], invj_s[s]
                nsqp = psBC.tile([128, P2], f32, tag="psbc",
                                 name=f"nsqp_{s}_{g}")
                gm = G4[g]
                pairs = [gm[i:i + 2] for i in range(0, len(gm), 2)]
                for pr in pairs:
                    trp2 = bpool.tile([128, 2, 2, P2], bf16,
                                      tag="trbf", bufs=10,
                                      name=f"trp2_{s}_{pr[0]}")
                    for k, m in enumerate(pr):
                        nc.gpsimd.dma_start(
                            trp2[:, k, :, :],
                            t_trfeat[m, s]
                            .rearrange("(h p) w hh -> p h (w hh)", p=128))
                        trbf_t[(s, m)] = trp2[:, k, :, :]
                    sq2 = bpool.tile([128, 2, 2, P2], bf16, tag="sqbf",
                                     name=f"sq2_{s}_{pr[0]}")
                    nc.vector.tensor_tensor(
                        out=sq2[:], in0=trp2[:], in1=trp2[:], op=ALU.mult)
                    for k, m in enumerate(pr):
                        sq_t[(s, m)] = sq2[:, k, :, :]
                for h in range(2):
                    for m in gm:
                        r = 32 * (m % 4)
                        nc.tensor.matmul(
                            nsqp[r:r + 1, :], ones_col16[:],
                            sq_t[(s, m)][:, h, :],
                            start=(h == 0), stop=(h == 1),
                            tile_position=(0, r))
                for m in gm:
                    sq_t.pop((s, m), None)
                nsqsb = smpool.tile([128, P2], f32, tag="nsqsb")
                nc.vector.tensor_copy(nsqsb[:], nsqp[:])
                m0, nmg = gm[0], len(gm)
                nc.sync.dma_start(
                    nrm_stage[m0:m0 + nmg, 0:P2],
                    nsqsb[0:(nmg - 1) * 32 + 1:32, :])
                nc.sync.dma_start(
                    invj[:, m0:m0 + nmg, :],
                    nrm_stage[m0:m0 + nmg].rearrange("m (q p) -> p m q",
                                                     p=128))
                xv = invj[:, m0:m0 + nmg, :]
                nw = smpool.tile([128, 3, 4, 4], f32, tag="nw",
                                 name=f"nw_{s}_{g}")
                xh = nw[:, 0, 0:nmg, :]
                yv = nw[:, 1, 0:nmg, :]
                tv = nw[:, 2, 0:nmg, :]
                nc.vector.tensor_scalar_mul(xh, xv, 0.5)
                nc.vector.tensor_scalar(
                    out=yv.bitcast(i32), in0=xv.bitcast(i32),
                    scalar1=1, scalar2=None, op0=ALU.logical_shift_right)
                nc.vector.tensor_scalar(
                    out=yv.bitcast(i32), in0=yv.bitcast(i32),
                    scalar1=-1, scalar2=0x5F3759DF,
                    op0=ALU.mult, op1=ALU.add)
                for _ in range(2):
                    nc.vector.tensor_tensor(out=tv, in0=yv, in1=yv,
                                            op=ALU.mult)
                    nc.vector.tensor_tensor(out=tv, in0=tv, in1=xh,
                                            op=ALU.mult)
                    nc.vector.tensor_scalar(
                        out=tv, in0=tv, scalar1=-1.0, scalar2=1.5,
                        op0=ALU.mult, op1=ALU.add)
                    nc.vector.tensor_tensor(out=yv, in0=yv, in1=tv,
                                            op=ALU.mult)
                nc.vector.tensor_copy(xv, yv)

            def emit_aggs(s, g, ezs):
                ldj, nd_stage = ldj_s[s], nd_stage_s[s]
                agp = psBC.tile([128, P2], f32, tag="psbc",
                                name=f"agp_{s}_{g}")
                for q in range(4):
                    pq = JC[q]
                    for m in G4[g]:
                        r = 32 * (m % 4)
                        nc.tensor.matmul(
                            agp[r:r + 2, :], ldj[0:pq, m, q, :],
                            ezs[q][m][0:pq, :],
                            start=(q == 0), stop=(q == 3),
                            tile_position=(0, r))
                ndsb = smpool.tile([128, P2], f32, tag="ndsb")
                nc.vector.tensor_copy(ndsb[:], agp[:])
                m0, nmg = G4[g][0], len(G4[g])
                nc.sync.dma_start(
                    nd_stage[0, m0:m0 + nmg, :],
                    ndsb[0:(nmg - 1) * 32 + 1:32, :])
                nc.sync.dma_start(
                    nd_stage[1, m0:m0 + nmg, :],
                    ndsb[1:(nmg - 1) * 32 + 2:32, :])

            def phase2(s, g):
                te_n, invj = te_n_s[s], invj_s[s]
                ezs = {}
                for q in range(4):
                    pq = JC[q]
                    j0 = 128 * q
                    ez_t = {}
                    for m in G4[g]:
                        trbf = trbf_t[(s, m)]
                        st = psA.tile([128, P2], f32, tag="psa",
                                      name=f"st_{s}_{g}_{q}_{m}")
                        for h in range(2):
                            nc.tensor.matmul(
                                st[0:pq, :], trbf[:, h, j0:j0 + pq],
                                te_n[:, h, :],
                                start=(h == 0), stop=(h == 1))
                        ez = ezpool.tile([128, P2], bf16, tag="ez",
                                         bufs=24, name=f"ez_{s}_{g}_{q}_{m}")
                        nc.scalar.activation(
                            ez[0:pq, :], st[0:pq, :], AF.Exp,
                            scale=invj[0:pq, m, q:q + 1])
                        ez_t[m] = ez
                    ezs[q] = ez_t
                if pending[0] is not None:
                    emit_aggs(*pending[0])
                pending[0] = (s, g, ezs)
                for m in G4[g]:
                    trbf_t.pop((s, m), None)

            def division(s):
                nd_stage = nd_stage_s[s]
                numt = spool.tile([121, 120], f32, tag=f"numt{s}",
                                  name=f"numt{s}")
                nc.sync.dma_start(
                    numt[:], nd_stage[0].rearrange("m j -> (m j)")
                    .rearrange("(p x) -> p x", p=121))
                dent = spool.tile([121, 120], f32, tag=f"dent{s}",
                                  name=f"dent{s}")
                nc.sync.dma_start(
                    dent[:], nd_stage[1].rearrange("m j -> (m j)")
                    .rearrange("(p x) -> p x", p=121))
                rden = spool.tile([121, 120], f32, tag=f"rden{s}",
                                  name=f"rden{s}")
                nc.vector.reciprocal(rden[:], dent[:])
                pdq = spool.tile([121, 120], f32, tag=f"pdq{s}",
                                 name=f"pdq{s}")
                nc.vector.tensor_tensor(out=pdq[:], in0=numt[:],
                                        in1=rden[:], op=ALU.mult)
                nc.sync.dma_start(
                    pd_stage_s[s][:].rearrange("m j -> (m j)")
                    .rearrange("(p x) -> p x", p=121), pdq[:])

            def tail(s):
                # read pmt_down as (j_row, m, k_col): 88-byte runs
                pdx = spool.tile([WF, NMEM * WF], f32, tag=f"pdx{s}",
                                 name=f"pdx{s}")
                nc.sync.dma_start(
                    pdx[:],
                    pd_stage_s[s][:].rearrange("m (j k) -> j m k", k=WF))
                d1a = psD.tile([WL, 512], f32, tag="d1a",
                               name=f"d1a_{s}")
                nc.tensor.matmul(d1a[:], upt32[:], pdx[:, 0:512])
                d1b = psD.tile([WL, NMEM * WF - 512], f32, tag="d1b",
                               name=f"d1b_{s}")
                nc.tensor.matmul(d1b[:], upt32[:], pdx[:, 512:])
                d1s = spool.tile([WL, NMEM, WF], bf16, tag="d1s")
                d1f = d1s[:].rearrange("l m j -> l (m j)")
                nc.vector.tensor_copy(d1f[:, 0:512], d1a[:])
                nc.vector.tensor_copy(d1f[:, 512:], d1b[:])
                d1t = spool.tile([WF, NMEM, WL], bf16, tag="d1t")
                s1 = spool.tile([WL, HL], f32, tag="s1", name=f"s1_{s}")
                s2 = spool.tile([WL, HL], f32, tag="s2", name=f"s2_{s}")
                for gg in range(6):
                    m0 = 5 * gg
                    for m in range(m0, m0 + 5):
                        trp = psD.tile([WF, WL], bf16,
                                       tag=("d1a" if m % 2 else "d1b"),
                                       name=f"trp_{s}_{m}")
                        nc.tensor.transpose(trp[:], d1s[:, m, :], ident[:])
                        nc.vector.tensor_copy(d1t[:, m, :], trp[:])
                    d2 = psA.tile([WL, 440], f32, tag="psa",
                                  name=f"d2_{s}_{gg}")
                    nc.tensor.matmul(
                        d2[:], upt16[:],
                        d1t[:, m0:m0 + 5, :].rearrange("k m a -> k (m a)"))
                    d2c = smpool.tile([WL, 440], f32, tag="d2c")
                    nc.vector.tensor_copy(d2c[:], d2[:])
                    d2v = d2c[:].rearrange("b (m a) -> b a m", m=5)
                    sqg = smpool.tile([WL, 440], f32, tag="sqg")
                    nc.vector.tensor_tensor(out=sqg[:], in0=d2c[:],
                                            in1=d2c[:], op=ALU.mult)
                    if gg == 0:
                        nc.vector.tensor_reduce(
                            s1[:], d2v, axis=AX.X, op=ALU.add)
                        nc.vector.tensor_reduce(
                            s2[:], sqg[:].rearrange("b (m a) -> b a m", m=5),
                            axis=AX.X, op=ALU.add)
                    else:
                        p1 = smpool.tile([WL, HL], f32, tag="p1")
                        nc.vector.tensor_reduce(
                            p1[:], d2v, axis=AX.X, op=ALU.add)
                        nc.vector.tensor_tensor(out=s1[:], in0=s1[:],
                                                in1=p1[:], op=ALU.add)
                        p2 = smpool.tile([WL, HL], f32, tag="p2")
                        nc.vector.tensor_reduce(
                            p2[:], sqg[:].rearrange("b (m a) -> b a m", m=5),
                            axis=AX.X, op=ALU.add)
                        nc.vector.tensor_tensor(out=s2[:], in0=s2[:],
                                                in1=p2[:], op=ALU.add)

                mean = spool.tile([WL, HL], f32, tag=f"mean{s}",
                                  name=f"mean{s}")
                nc.vector.tensor_scalar_mul(mean[:], s1[:], 1.0 / NMEM)
                ms = spool.tile([WL, HL], f32, tag="ms")
                nc.vector.tensor_tensor(out=ms[:], in0=mean[:], in1=mean[:],
                                        op=ALU.mult)
                v1 = spool.tile([WL, HL], f32, tag="v1")
                nc.vector.tensor_scalar_mul(v1[:], s2[:], 1.0 / (NMEM - 1))
                v2 = spool.tile([WL, HL], f32, tag="v2")
                nc.vector.tensor_scalar_mul(v2[:], ms[:], NMEM / (NMEM - 1.0))
                var = spool.tile([WL, HL], f32, tag="var")
                nc.vector.tensor_tensor(out=var[:], in0=v1[:], in1=v2[:],
                                        op=ALU.subtract)
                vp1 = spool.tile([WL, HL], f32, tag="vp1")
                nc.vector.tensor_scalar_add(vp1[:], var[:], 1.0)
                rv = spool.tile([WL, HL], f32, tag=f"rv{s}", name=f"rv{s}")
                nc.vector.reciprocal(rv[:], vp1[:])
                mean_s[s], rv_s[s] = mean, rv

            warm = spool.tile([1, 1], f32, tag="warm")
            nc.scalar.activation(warm[:], temp_t[:], AF.Exp, scale=0.001)

            NSTEP = len(STEPS)
            for i in range(NSTEP + LA):
                if i < NSTEP:
                    phase1(*STEPS[i])
                if i >= LA:
                    phase2(*STEPS[i - LA])
                if i - LA == 2 * len(G4) - 2:
                    division(0)
            if pending[0] is not None:
                emit_aggs(*pending[0])
                pending[0] = None
            division(1)
            tail(0)
            tail(1)

            # =========== deferred certainty + output ===========
            nalpha = cpool.tile([WL, 1], f32)
            nc.vector.memset(nalpha[:], -ALPHA)
            for s in range(SEQ_LOC):
                cert = spool.tile([WL, HL], f32, tag=f"cert{s}",
                                  name=f"cert{s}")
                nc.scalar.activation(cert[:], rv_s[s][:], AF.Exp,
                                     scale=ALPHA, bias=nalpha[:])
                o1 = spool.tile([WL, HL], f32, tag=f"o1{s}", name=f"o1{s}")
                nc.vector.tensor_tensor(out=o1[:], in0=cert[:],
                                        in1=mean_s[s][:], op=ALU.mult)
                o1p = psD.tile([WL, HL], f32, tag="d1a", name=f"o1p{s}")
                nc.tensor.transpose(o1p[:], o1[:], ident32[:])
                o1t = spool.tile([WL, HL], f32, tag=f"o1t{s}",
                                 name=f"o1t{s}")
                nc.vector.tensor_copy(o1t[:], o1p[:])
                o2 = spool.tile([WL, HL], f32, tag=f"o2{s}", name=f"o2{s}")
                nc.vector.tensor_tensor(out=o2[:], in0=o1t[:],
                                        in1=ts_s[s][:], op=ALU.add)
                nc.sync.dma_start(t_out[s], o2[:])

    nc.compile()
    return nc


def _get_nc():
    if "nc" not in _CACHE:
        _CACHE["nc"] = _build()
    return _CACHE["nc"]


def _bf16(a):
    import ml_dtypes
    return np.ascontiguousarray(a).astype(ml_dtypes.bfloat16)


def _run(test_scores, train_labels, test_feat, train_feats, softmax_temp,
         trace=False):
    from concourse.bass_utils import run_bass_kernel_spmd

    test_scores = np.ascontiguousarray(test_scores, np.float32)
    train_labels = np.ascontiguousarray(train_labels, np.float32)
    test_feat = np.ascontiguousarray(test_feat, np.float32)
    train_feats = np.ascontiguousarray(train_feats, np.float32)
    temp = np.ascontiguousarray(softmax_temp, np.float32).reshape(1)

    te = test_feat[0].reshape(NSEQ, C, P2)
    nrm = np.sqrt((te * te).sum(axis=1, keepdims=True))
    ten = _bf16(temp[0] * te / nrm)
    import concourse.mybir as mybir
    ten8 = (temp[0] * te / nrm).astype(mybir.dt.np(mybir.dt.float8e4))

    lab = train_labels.reshape(NMEM * NSEQ, WL, HL)
    ld = (_DN @ lab @ _DN.T).reshape(NMEM, NSEQ, P2)
    ldj = np.zeros((NSEQ, 128, NMEM, 4, 2), np.float32)
    for q in range(4):
        pq = JC[q]
        ldj[:, 0:pq, :, q, 0] = ld[:, :, 128 * q:128 * q + pq].transpose(
            1, 2, 0)
        ldj[:, 0:pq, :, q, 1] = 1.0
    ldj = _bf16(ldj)

    in_maps = []
    for c in range(NCORES):
        sl = slice(SEQ_LOC * c, SEQ_LOC * (c + 1))
        in_maps.append({
            "t_scores": test_scores[0, sl],
            "t_ldj": ldj[sl],
            "t_ten": ten[sl],
            "t_ten8": ten8[sl],
            "t_trfeat": np.ascontiguousarray(train_feats[:, sl]),
            "t_temp": temp,
            "t_upt32": np.ascontiguousarray(_UP.T),
            "t_upt16": _bf16(_UP.T),
            "t_ident": _bf16(np.eye(WL, dtype=np.float32)),
            "t_ident32": np.eye(WL, dtype=np.float32),
        })
    nc = _get_nc()
    res = run_bass_kernel_spmd(nc, in_maps, list(range(NCORES)), trace=trace)
    out = np.empty((1, NSEQ, WL, HL), np.float32)
    for c in range(NCORES):
        out[0, SEQ_LOC * c:SEQ_LOC * (c + 1)] = res.results[c]["t_out"]
    return out, res


def kernel(test_scores, train_labels, test_feat, train_feats, softmax_temp):
    out, _ = _run(test_scores, train_labels, test_feat, train_feats,
                  softmax_temp, trace=False)
    return out


# revision 31
# speedup vs baseline: 1.0882x; 1.0198x over previous
"""nn_AttentionAverageStdScalingModule — Trainium2 Bass/Tile kernel.

Contract: kernel(**inputs) takes FULL unsharded inputs and returns the FULL
output (1, 16, 88, 88) f32.  The nseq axis (16) is sharded 2-per-core across
8 NeuronCores; each core runs an identical program on its 2 sequences.

Per sequence s:
  te_n = softmax_temp * test_feat[:,s]/||cols||             (256, 484)
  for each memory m: simT[j,i] = sum_c tr[c,j]*te_n[c,i]    (484j, 484i)
      ez = exp(simT * rsqrt(nsq_j))   <- per-partition scale on ScalarE
      [num;den][i] = [labels_down_m; ones]^T @ ez           (2, 484)
  pmt_down = num/den; pmt = UP @ pmt_down @ UP^T  (bilinear upsample)
  mean/unbiased-std over m, certainty = exp(A/(1+std^2)-A)
  out = certainty*mean + test_scores

Engine split: PE does sim/aggregation/norm-sums/resampling matmuls (bf16),
ScalarE does only Exp (no LUT switches), VectorE does squares/stats and a
bit-trick rsqrt (Newton x2), GpSimd does the fp32->bf16 casts.  Memory-
sums land 4-memories-per-PSUM-bank at 32-aligned partitions so copies and
DRAM relayout bounces are batched.
"""

import numpy as np

NMEM, NSEQ, C, WF, HF = 30, 16, 256, 22, 22
WL, HL = 88, 88
P2 = WF * HF            # 484
NCORES = 8
SEQ_LOC = NSEQ // NCORES  # 2
ALPHA = 20.0
JC = [128, 128, 128, 100]   # j-chunk sizes covering 484
G4 = [list(range(4 * g, min(4 * g + 4, NMEM))) for g in range(8)]
# rsqrt batches (in units of g4 groups): ramp up so exp can start early
BATCHES = [[0], [1, 2], [3, 4], [5, 6, 7]]


def _resize_matrix(n_in: int, n_out: int) -> np.ndarray:
    """Row-stochastic 1-D bilinear resize matrix (half-pixel centers,
    out-of-range taps dropped + renormalized) matching
    jax.image.resize(method='bilinear', antialias=False)."""
    M = np.zeros((n_out, n_in), np.float64)
    scale = n_in / n_out
    for i in range(n_out):
        x = (i + 0.5) * scale - 0.5
        x0 = int(np.floor(x))
        for tap, w in ((x0, 1.0 - (x - x0)), (x0 + 1, x - x0)):
            if 0 <= tap < n_in and w > 0.0:
                M[i, tap] += w
        s = M[i].sum()
        if s > 0:
            M[i] /= s
    return M.astype(np.float32)


_DN = _resize_matrix(WL, WF)   # (22, 88)  downsample
_UP = _resize_matrix(WF, WL)   # (88, 22)  upsample

_CACHE = {}


def _build():
    import concourse.bacc as bacc
    import concourse.mybir as mybir
    from concourse import tile

    f32 = mybir.dt.float32
    bf16 = mybir.dt.bfloat16
    fp8 = mybir.dt.float8e4
    i32 = mybir.dt.int32
    AF = mybir.ActivationFunctionType
    ALU = mybir.AluOpType
    AX = mybir.AxisListType

    nc = bacc.Bacc("TRN2", target_bir_lowering=False, debug=False,
                   num_devices=NCORES)

    t_scores = nc.dram_tensor("t_scores", [SEQ_LOC, WL, HL], f32,
                              kind="ExternalInput")
    t_ldj = nc.dram_tensor("t_ldj", [SEQ_LOC, 128, NMEM, 4, 2], bf16,
                           kind="ExternalInput")
    t_ten = nc.dram_tensor("t_ten", [SEQ_LOC, C, P2], bf16,
                           kind="ExternalInput")
    t_ten8 = nc.dram_tensor("t_ten8", [SEQ_LOC, C, P2], fp8,
                            kind="ExternalInput")
    t_trfeat = nc.dram_tensor("t_trfeat", [NMEM, SEQ_LOC, C, WF, HF], f32,
                              kind="ExternalInput")
    t_temp = nc.dram_tensor("t_temp", [1], f32, kind="ExternalInput")
    t_upt32 = nc.dram_tensor("t_upt32", [WF, WL], f32, kind="ExternalInput")
    t_upt16 = nc.dram_tensor("t_upt16", [WF, WL], bf16, kind="ExternalInput")
    t_ident = nc.dram_tensor("t_ident", [WL, WL], bf16, kind="ExternalInput")
    t_ident32 = nc.dram_tensor("t_ident32", [WL, WL], f32,
                               kind="ExternalInput")
    t_out = nc.dram_tensor("t_out", [SEQ_LOC, WL, HL], f32,
                           kind="ExternalOutput")

    with tile.TileContext(nc) as tc:
        with (
            tc.tile_pool(name="const", bufs=1) as cpool,
            tc.tile_pool(name="seq", bufs=1) as spool,
            tc.tile_pool(name="big", bufs=3) as bpool,
            tc.tile_pool(name="ez", bufs=4) as ezpool,
            tc.tile_pool(name="sm", bufs=2) as smpool,
            tc.tile_pool(name="dram", bufs=1, space="DRAM") as dpool,
            tc.tile_pool(name="psA", bufs=4, space="PSUM") as psA,
            tc.tile_pool(name="psBC", bufs=2, space="PSUM") as psBC,
            tc.tile_pool(name="psD", bufs=1, space="PSUM") as psD,
        ):
            # ---- constants ----
            ones_col32 = cpool.tile([128, 1], f32)
            nc.vector.memset(ones_col32[:], 1.0)
            ones_col16 = cpool.tile([128, 1], bf16)
            nc.vector.memset(ones_col16[:], 1.0)
            ones_row32 = cpool.tile([1, 128], f32)
            nc.vector.memset(ones_row32[:], 1.0)
            upt32 = cpool.tile([WF, WL], f32)      # UP^T (22, 88)
            nc.sync.dma_start(upt32[:], t_upt32[:])
            upt16 = cpool.tile([WF, WL], bf16)
            nc.sync.dma_start(upt16[:], t_upt16[:])
            ident = cpool.tile([WL, WL], bf16)
            nc.sync.dma_start(ident[:], t_ident[:])
            ident32 = cpool.tile([WL, WL], f32)
            nc.sync.dma_start(ident32[:], t_ident32[:])
            temp_t = cpool.tile([1, 1], f32)
            nc.sync.dma_start(temp_t[:], t_temp[:])

            te_n_s, ldj_s, invj_s, labs_s = {}, {}, {}, {}
            nrm_stage_s, nd_stage_s, pd_stage_s = {}, {}, {}
            mean_s, rv_s, ts_s, pdx_s = {}, {}, {}, {}

            # =========== prep for BOTH sequences up front ===========
            for s in range(SEQ_LOC):
                te_n = spool.tile([128, 2, P2], bf16, tag=f"te_n{s}",
                                  name=f"te_n{s}")
                nc.sync.dma_start(
                    te_n[:],
                    t_ten[s].rearrange("(h p) j -> p h j", p=128))
                te_n_s[s] = te_n
                ldj = spool.tile([128, NMEM, 4, 2], bf16,
                                 tag=f"ldj{s}", name=f"ldj{s}")
                nc.sync.dma_start(ldj[:], t_ldj[s])
                ldj_s[s] = ldj
                nrm_stage_s[s] = dpool.tile([NMEM, 512], f32,
                                            tag=f"nrm_stage{s}",
                                            name=f"nrm_stage{s}")
                nd_stage_s[s] = dpool.tile([2, NMEM, P2], f32,
                                           tag=f"nd_stage{s}",
                                           name=f"nd_stage{s}")
                pd_stage_s[s] = dpool.tile([NMEM, P2], f32,
                                           tag=f"pd_stage{s}",
                                           name=f"pd_stage{s}")
                invj_s[s] = spool.tile([128, NMEM, 4], f32, tag=f"invj{s}",
                                       name=f"invj{s}")
                ts = spool.tile([WL, HL], f32, tag=f"ts{s}", name=f"ts{s}")
                nc.sync.dma_start(ts[:], t_scores[s])
                ts_s[s] = ts

            # =========== flat 16-group pipeline across both seqs ======
            trbf_t, sq_t, tr8_t = {}, {}, {}
            agp_t = {}
            pending = [None]
            LA = 2
            STEPS = [(s, g) for g in range(len(G4)) for s in range(SEQ_LOC)]

            def phase1(s, g):
                nrm_stage, invj = nrm_stage_s[s], invj_s[s]
                nsqp = psBC.tile([128, P2], f32, tag="psbc",
                                 name=f"nsqp_{s}_{g}")
                gm = G4[g]
                pairs = [gm[i:i + 2] for i in range(0, len(gm), 2)]
                for pr in pairs:
                    trp2 = bpool.tile([128, 2, 2, P2], bf16,
                                      tag="trbf", bufs=10,
                                      name=f"trp2_{s}_{pr[0]}")
                    for k, m in enumerate(pr):
                        nc.gpsimd.dma_start(
                            trp2[:, k, :, :],
                            t_trfeat[m, s]
                            .rearrange("(h p) w hh -> p h (w hh)", p=128))
                        trbf_t[(s, m)] = trp2[:, k, :, :]
                    sq2 = bpool.tile([128, 2, 2, P2], bf16, tag="sqbf",
                                     name=f"sq2_{s}_{pr[0]}")
                    nc.vector.tensor_tensor(
                        out=sq2[:], in0=trp2[:], in1=trp2[:], op=ALU.mult)
                    for k, m in enumerate(pr):
                        sq_t[(s, m)] = sq2[:, k, :, :]
                for h in range(2):
                    for m in gm:
                        r = 32 * (m % 4)
                        nc.tensor.matmul(
                            nsqp[r:r + 1, :], ones_col16[:],
                            sq_t[(s, m)][:, h, :],
                            start=(h == 0), stop=(h == 1),
                            tile_position=(0, r))
                for m in gm:
                    sq_t.pop((s, m), None)
                nsqsb = smpool.tile([128, P2], f32, tag="nsqsb")
                nc.vector.tensor_copy(nsqsb[:], nsqp[:])
                m0, nmg = gm[0], len(gm)
                nc.sync.dma_start(
                    nrm_stage[m0:m0 + nmg, 0:P2],
                    nsqsb[0:(nmg - 1) * 32 + 1:32, :])
                nc.sync.dma_start(
                    invj[:, m0:m0 + nmg, :],
                    nrm_stage[m0:m0 + nmg].rearrange("m (q p) -> p m q",
                                                     p=128))
                xv = invj[:, m0:m0 + nmg, :]
                nw = smpool.tile([128, 3, 4, 4], f32, tag="nw",
                                 name=f"nw_{s}_{g}")
                xh = nw[:, 0, 0:nmg, :]
                yv = nw[:, 1, 0:nmg, :]
                tv = nw[:, 2, 0:nmg, :]
                nc.vector.tensor_scalar_mul(xh, xv, 0.5)
                nc.vector.tensor_scalar(
                    out=yv.bitcast(i32), in0=xv.bitcast(i32),
                    scalar1=1, scalar2=None, op0=ALU.logical_shift_right)
                nc.vector.tensor_scalar(
                    out=yv.bitcast(i32), in0=yv.bitcast(i32),
                    scalar1=-1, scalar2=0x5F3759DF,
                    op0=ALU.mult, op1=ALU.add)
                for _ in range(2):
                    nc.vector.tensor_tensor(out=tv, in0=yv, in1=yv,
                                            op=ALU.mult)
                    nc.vector.tensor_tensor(out=tv, in0=tv, in1=xh,
                                            op=ALU.mult)
                    nc.vector.tensor_scalar(
                        out=tv, in0=tv, scalar1=-1.0, scalar2=1.5,
                        op0=ALU.mult, op1=ALU.add)
                    nc.vector.tensor_tensor(out=yv, in0=yv, in1=tv,
                                            op=ALU.mult)
                nc.vector.tensor_copy(xv, yv)

            def emit_aggs(s, g, ezs):
                ldj, nd_stage = ldj_s[s], nd_stage_s[s]
                agp = psBC.tile([128, P2], f32, tag="psbc",
                                name=f"agp_{s}_{g}")
                for q in range(4):
                    pq = JC[q]
                    for m in G4[g]:
                        r = 32 * (m % 4)
                        nc.tensor.matmul(
                            agp[r:r + 2, :], ldj[0:pq, m, q, :],
                            ezs[q][m][0:pq, :],
                            start=(q == 0), stop=(q == 3),
                            tile_position=(0, r))
                ndsb = smpool.tile([128, P2], f32, tag="ndsb")
                nc.vector.tensor_copy(ndsb[:], agp[:])
                m0, nmg = G4[g][0], len(G4[g])
                nc.sync.dma_start(
                    nd_stage[0, m0:m0 + nmg, :],
                    ndsb[0:(nmg - 1) * 32 + 1:32, :])
                nc.sync.dma_start(
                    nd_stage[1, m0:m0 + nmg, :],
                    ndsb[1:(nmg - 1) * 32 + 2:32, :])

            def phase2(s, g):
                te_n, invj = te_n_s[s], invj_s[s]
                ezs = {}
                for q in range(4):
                    pq = JC[q]
                    j0 = 128 * q
                    ez_t = {}
                    for m in G4[g]:
                        trbf = trbf_t[(s, m)]
                        st = psA.tile([128, P2], f32, tag="psa",
                                      name=f"st_{s}_{g}_{q}_{m}")
                        for h in range(2):
                            nc.tensor.matmul(
                                st[0:pq, :], trbf[:, h, j0:j0 + pq],
                                te_n[:, h, :],
                                start=(h == 0), stop=(h == 1))
                        ez = ezpool.tile([128, P2], bf16, tag="ez",
                                         bufs=24, name=f"ez_{s}_{g}_{q}_{m}")
                        nc.scalar.activation(
                            ez[0:pq, :], st[0:pq, :], AF.Exp,
                            scale=invj[0:pq, m, q:q + 1])
                        ez_t[m] = ez
                    ezs[q] = ez_t
                if pending[0] is not None:
                    emit_aggs(*pending[0])
                pending[0] = (s, g, ezs)
                for m in G4[g]:
                    trbf_t.pop((s, m), None)

            def division(s):
                nd_stage = nd_stage_s[s]
                numt = spool.tile([121, 120], f32, tag=f"numt{s}",
                                  name=f"numt{s}")
                nc.sync.dma_start(
                    numt[:], nd_stage[0].rearrange("m j -> (m j)")
                    .rearrange("(p x) -> p x", p=121))
                dent = spool.tile([121, 120], f32, tag=f"dent{s}",
                                  name=f"dent{s}")
                nc.sync.dma_start(
                    dent[:], nd_stage[1].rearrange("m j -> (m j)")
                    .rearrange("(p x) -> p x", p=121))
                rden = spool.tile([121, 120], f32, tag=f"rden{s}",
                                  name=f"rden{s}")
                nc.vector.reciprocal(rden[:], dent[:])
                pdq = spool.tile([121, 120], f32, tag=f"pdq{s}",
                                 name=f"pdq{s}")
                nc.vector.tensor_tensor(out=pdq[:], in0=numt[:],
                                        in1=rden[:], op=ALU.mult)
                nc.sync.dma_start(
                    pd_stage_s[s][:].rearrange("m j -> (m j)")
                    .rearrange("(p x) -> p x", p=121), pdq[:])

            def tail(s):
                # read pmt_down as (j_row, m, k_col): 88-byte runs
                pdx = spool.tile([WF, NMEM * WF], f32, tag=f"pdx{s}",
                                 name=f"pdx{s}")
                nc.sync.dma_start(
                    pdx[:],
                    pd_stage_s[s][:].rearrange("m (j k) -> j m k", k=WF))
                d1a = psD.tile([WL, 512], f32, tag="d1a",
                               name=f"d1a_{s}")
                nc.tensor.matmul(d1a[:], upt32[:], pdx[:, 0:512])
                d1b = psD.tile([WL, NMEM * WF - 512], f32, tag="d1b",
                               name=f"d1b_{s}")
                nc.tensor.matmul(d1b[:], upt32[:], pdx[:, 512:])
                d1s = spool.tile([WL, NMEM, WF], bf16, tag="d1s")
                d1f = d1s[:].rearrange("l m j -> l (m j)")
                nc.vector.tensor_copy(d1f[:, 0:512], d1a[:])
                nc.vector.tensor_copy(d1f[:, 512:], d1b[:])
                d1t = spool.tile([WF, NMEM, WL], bf16, tag="d1t")
                s1 = spool.tile([WL, HL], f32, tag="s1", name=f"s1_{s}")
                s2 = spool.tile([WL, HL], f32, tag="s2", name=f"s2_{s}")
                for gg in range(6):
                    m0 = 5 * gg
                    for m in range(m0, m0 + 5):
                        trp = psA.tile([WF, WL], bf16, tag="psa",
                                       name=f"trp_{s}_{m}")
                        nc.tensor.transpose(trp[:], d1s[:, m, :], ident[:])
                        nc.vector.tensor_copy(d1t[:, m, :], trp[:])
                    d2 = psA.tile([WL, 440], f32, tag="psa",
                                  name=f"d2_{s}_{gg}")
                    nc.tensor.matmul(
                        d2[:], upt16[:],
                        d1t[:, m0:m0 + 5, :].rearrange("k m a -> k (m a)"))
                    d2c = smpool.tile([WL, 440], f32, tag="d2c")
                    nc.vector.tensor_copy(d2c[:], d2[:])
                    d2v = d2c[:].rearrange("b (m a) -> b a m", m=5)
                    sqg = smpool.tile([WL, 440], f32, tag="sqg")
                    nc.vector.tensor_tensor(out=sqg[:], in0=d2c[:],
                                            in1=d2c[:], op=ALU.mult)
                    if gg == 0:
                        nc.vector.tensor_reduce(
                            s1[:], d2v, axis=AX.X, op=ALU.add)
                        nc.vector.tensor_reduce(
                            s2[:], sqg[:].rearrange("b (m a) -> b a m", m=5),
                            axis=AX.X, op=ALU.add)
                    else:
                        p1 = smpool.tile([WL, HL], f32, tag="p1")
                        nc.vector.tensor_reduce(
                            p1[:], d2v, axis=AX.X, op=ALU.add)
                        nc.vector.tensor_tensor(out=s1[:], in0=s1[:],
                                                in1=p1[:], op=ALU.add)
                        p2 = smpool.tile([WL, HL], f32, tag="p2")
                        nc.vector.tensor_reduce(
                            p2[:], sqg[:].rearrange("b (m a) -> b a m", m=5),
                            axis=AX.X, op=ALU.add)
                        nc.vector.tensor_tensor(out=s2[:], in0=s2[:],
                                                in1=p2[:], op=ALU.add)

                mean = spool.tile([WL, HL], f32, tag=f"mean{s}",
                                  name=f"mean{s}")
                nc.vector.tensor_scalar_mul(mean[:], s1[:], 1.0 / NMEM)
                ms = spool.tile([WL, HL], f32, tag="ms")
                nc.vector.tensor_tensor(out=ms[:], in0=mean[:], in1=mean[:],
                                        op=ALU.mult)
                v1 = spool.tile([WL, HL], f32, tag="v1")
                nc.vector.tensor_scalar_mul(v1[:], s2[:], 1.0 / (NMEM - 1))
                v2 = spool.tile([WL, HL], f32, tag="v2")
                nc.vector.tensor_scalar_mul(v2[:], ms[:], NMEM / (NMEM - 1.0))
                var = spool.tile([WL, HL], f32, tag="var")
                nc.vector.tensor_tensor(out=var[:], in0=v1[:], in1=v2[:],
                                        op=ALU.subtract)
                vp1 = spool.tile([WL, HL], f32, tag="vp1")
                nc.vector.tensor_scalar_add(vp1[:], var[:], 1.0)
                rv = spool.tile([WL, HL], f32, tag=f"rv{s}", name=f"rv{s}")
                nc.vector.reciprocal(rv[:], vp1[:])
                mean_s[s], rv_s[s] = mean, rv

            warm = spool.tile([1, 1], f32, tag="warm")
            nc.scalar.activation(warm[:], temp_t[:], AF.Exp, scale=0.001)

            NSTEP = len(STEPS)
            for i in range(NSTEP + LA):
                if i < NSTEP:
                    phase1(*STEPS[i])
                if i >= LA:
                    phase2(*STEPS[i - LA])
                if i - LA == 2 * len(G4) - 2:
                    division(0)
                if i - LA == 2 * len(G4) - 1:
                    tail(0)
            if pending[0] is not None:
                emit_aggs(*pending[0])
                pending[0] = None
            division(1)
            tail(1)

            # =========== deferred certainty + output ===========
            nalpha = cpool.tile([WL, 1], f32)
            nc.vector.memset(nalpha[:], -ALPHA)
            for s in range(SEQ_LOC):
                cert = spool.tile([WL, HL], f32, tag=f"cert{s}",
                                  name=f"cert{s}")
                nc.scalar.activation(cert[:], rv_s[s][:], AF.Exp,
                                     scale=ALPHA, bias=nalpha[:])
                o1 = spool.tile([WL, HL], f32, tag=f"o1{s}", name=f"o1{s}")
                nc.vector.tensor_tensor(out=o1[:], in0=cert[:],
                                        in1=mean_s[s][:], op=ALU.mult)
                o1p = psD.tile([WL, HL], f32, tag="d1a", name=f"o1p{s}")
                nc.tensor.transpose(o1p[:], o1[:], ident32[:])
                o1t = spool.tile([WL, HL], f32, tag=f"o1t{s}",
                                 name=f"o1t{s}")
                nc.vector.tensor_copy(o1t[:], o1p[:])
                o2 = spool.tile([WL, HL], f32, tag=f"o2{s}", name=f"o2{s}")
                nc.vector.tensor_tensor(out=o2[:], in0=o1t[:],
                                        in1=ts_s[s][:], op=ALU.add)
                nc.sync.dma_start(t_out[s], o2[:])

    nc.compile()
    return nc


def _get_nc():
    if "nc" not in _CACHE:
        _CACHE["nc"] = _build()
    return _CACHE["nc"]


def _bf16(a):
    import ml_dtypes
    return np.ascontiguousarray(a).astype(ml_dtypes.bfloat16)


def _run(test_scores, train_labels, test_feat, train_feats, softmax_temp,
         trace=False):
    from concourse.bass_utils import run_bass_kernel_spmd

    test_scores = np.ascontiguousarray(test_scores, np.float32)
    train_labels = np.ascontiguousarray(train_labels, np.float32)
    test_feat = np.ascontiguousarray(test_feat, np.float32)
    train_feats = np.ascontiguousarray(train_feats, np.float32)
    temp = np.ascontiguousarray(softmax_temp, np.float32).reshape(1)

    te = test_feat[0].reshape(NSEQ, C, P2)
    nrm = np.sqrt((te * te).sum(axis=1, keepdims=True))
    ten = _bf16(temp[0] * te / nrm)
    import concourse.mybir as mybir
    ten8 = (temp[0] * te / nrm).astype(mybir.dt.np(mybir.dt.float8e4))

    lab = train_labels.reshape(NMEM * NSEQ, WL, HL)
    ld = (_DN @ lab @ _DN.T).reshape(NMEM, NSEQ, P2)
    ldj = np.zeros((NSEQ, 128, NMEM, 4, 2), np.float32)
    for q in range(4):
        pq = JC[q]
        ldj[:, 0:pq, :, q, 0] = ld[:, :, 128 * q:128 * q + pq].transpose(
            1, 2, 0)
        ldj[:, 0:pq, :, q, 1] = 1.0
    ldj = _bf16(ldj)

    in_maps = []
    for c in range(NCORES):
        sl = slice(SEQ_LOC * c, SEQ_LOC * (c + 1))
        in_maps.append({
            "t_scores": test_scores[0, sl],
            "t_ldj": ldj[sl],
            "t_ten": ten[sl],
            "t_ten8": ten8[sl],
            "t_trfeat": np.ascontiguousarray(train_feats[:, sl]),
            "t_temp": temp,
            "t_upt32": np.ascontiguousarray(_UP.T),
            "t_upt16": _bf16(_UP.T),
            "t_ident": _bf16(np.eye(WL, dtype=np.float32)),
            "t_ident32": np.eye(WL, dtype=np.float32),
        })
    nc = _get_nc()
    res = run_bass_kernel_spmd(nc, in_maps, list(range(NCORES)), trace=trace)
    out = np.empty((1, NSEQ, WL, HL), np.float32)
    for c in range(NCORES):
        out[0, SEQ_LOC * c:SEQ_LOC * (c + 1)] = res.results[c]["t_out"]
    return out, res


def kernel(test_scores, train_labels, test_feat, train_feats, softmax_temp):
    out, _ = _run(test_scores, train_labels, test_feat, train_feats,
                  softmax_temp, trace=False)
    return out
